# revision 68
# baseline (speedup 1.0000x reference)
"""Causal single-head attention (B=4, S=2048, D=1024) on 8 TRN2 NeuronCores.

Sharding: core c -> (batch b = c//2, half h = c%2). Every core runs the SAME
program: 8 query tiles of 128 rows whose padded causal key-lengths are
L_s = 256*(s+1) for s=0..7.  Core (b, h) takes global query rows
[256*s + 128*h, 256*s + 128*h + 128) of batch b for slot s.

All device operands are bf16 (quantized host-side; PSUM accumulates fp32).
X and W are shipped pre-transposed/reshaped as [P, DC, cols] so the
contraction dim d sits on the partitions for every projection matmul and the
device never transposes activations.

Phases: G^T -> attention; there is NO K or V projection on device.
Both sides are reassociated so every GEMM scales with this core's 1024
queries instead of the 2048 keys: scores = X_q (Wq Wk^T) X_k^T with the
weight-only product Wqk = Wq@Wk^T fused host-side (G^T = Wqk^T X_q^T is the
only projection phase), and out = ((attn @ X_v) @ W_v) / den (U^T).  The
per-core-pair duplicated K and V projections disappear entirely; X_k^T and
X_v are simply loaded resident.  Weights are prefetched one phase ahead, X streamed in 512-column blocks,
and the first K block runs dc-outer so compute starts as soon as the first
weight/X chunks land.  Attention computes scores TRANSPOSED (scores^T[k, q],
keys on partitions): exp output lands directly in the attn^T layout that both
the U^T matmul (rhs) and nothing else needs -- no PE transposes anywhere
(U^T is produced directly with the natural-layout X_v chunk as stationary).
The causal mask is two data-driven [128, 128] tiles (h-dependent) added to
the last two key tiles of each slot; the softmax denominator is an ap=1
matmul against a ones vector; 1/den is applied on the Y = U @ W_v copy-out
(ACT low half, DVE tensor_scalar high half, in parallel).  Slots run
longest-first and are software-pipelined at score-group granularity so the
PE never waits on an exp.

NOTE (hazard found empirically): interleaving start/stop matmul accumulation
chains across sub-regions of a single PSUM bank corrupts results on HW;
interleaving across distinct banks is fine.  Keep each sub-tile's ec-loop
contiguous.
"""

import numpy as np

import concourse.bacc as bacc
import concourse.mybir as mybir
import concourse.tile as tile
from concourse import bass_utils

B, S, D = 4, 2048, 1024
P = 128
DC = D // P          # 8 contraction chunks
EC = D // P          # 8 output-dim chunks
NSLOT = 8            # q tiles per core
NQ = NSLOT * P       # 1024 q rows per core
SCALE = 1.0 / float(np.sqrt(np.float32(S)))
NEG = -1.0e9

F32 = mybir.dt.float32
BF16 = mybir.dt.bfloat16


def build_attention_nc():
    nc = bacc.Bacc("TRN2", target_bir_lowering=False)

    xqT = nc.dram_tensor("xqT", [P, DC, NQ], BF16, kind="ExternalInput")
    xkT = nc.dram_tensor("xkT", [P, DC, S], BF16, kind="ExternalInput")
    xv_in = nc.dram_tensor("xv", [P, S // P, D], BF16, kind="ExternalInput")
    wqk = nc.dram_tensor("wqk", [P, DC, D], BF16, kind="ExternalInput")
    wv = nc.dram_tensor("wv", [P, DC, D], BF16, kind="ExternalInput")
    mask_a_in = nc.dram_tensor("mask_a", [P, P], BF16, kind="ExternalInput")
    mask_b_in = nc.dram_tensor("mask_b", [P, P], BF16, kind="ExternalInput")
    ones_in = nc.dram_tensor("ones", [P, 1], BF16, kind="ExternalInput")
    out = nc.dram_tensor("out", [NQ, D], BF16, kind="ExternalOutput")

    with tile.TileContext(nc) as tc:
        with (
            tc.tile_pool(name="res", bufs=1) as res,
            tc.tile_pool(name="psum", bufs=4, space="PSUM") as pp,
        ):
            kt_sb = res.tile([P, EC, S], BF16)      # K^T  [e, keys]
            xv_sb = res.tile([P, S // P, D], BF16)  # X_v  [keys, d] (natural)
            qt_sb = res.tile([P, EC, NQ], BF16)     # Q^T  [e, q]
            wv_rs = res.tile([P, DC, D], BF16)      # W_v resident for U@Wv
            mask_a = res.tile([P, P], BF16)
            mask_b = res.tile([P, P], BF16)
            ones_sb = res.tile([P, 1], BF16)

            # ============ projection phases (K^T, V, Q^T) ============
            # One PSUM pool for the whole kernel: projection tiles and the
            # attention score tiles share tag "ps" ([P, 512]), so there is no
            # pool-close drain between phases.
            with (
                tc.tile_pool(name="wp", bufs=2) as wp,
                tc.tile_pool(name="xp", bufs=3) as xp,
            ):
                def load_w(w_t, chunks=2, engs=(nc.gpsimd,)):
                    w_sb = wp.tile([P, DC, D], BF16, tag="w")
                    n = DC // chunks
                    for i in range(chunks):
                        engs[i % len(engs)].dma_start(
                            w_sb[:, i * n : (i + 1) * n],
                            w_t[:, i * n : (i + 1) * n],
                        )
                    return w_sb

                def load_x(x_t, c0, width, split=2, engs=(nc.sync,)):
                    xb = xp.tile([P, DC, width], BF16, tag="x")
                    n = DC // split
                    for i in range(split):
                        engs[i % len(engs)].dma_start(
                            xb[:, i * n : (i + 1) * n],
                            x_t[:, i * n : (i + 1) * n, c0 : c0 + width],
                        )
                    return xb

                # Startup: interleave the critical first loads across the
                # shared HWDGE (sync+scalar) and the SWDGE (gpsimd) domains so
                # the first matmul can issue ~3us in.  The G^T qb=1 pass runs
                # dc-outer so compute starts as soon as (wqk[0], xq[0]) land.
                wqk_sb = wp.tile([P, DC, D], BF16, tag="w")
                xb0 = xp.tile([P, DC, 512], BF16, tag="x")
                nc.sync.dma_start(wqk_sb[:, 0, 0:512], wqk[:, 0, 0:512])
                nc.scalar.dma_start(xb0[:, 0], xqT[:, 0, 512:NQ])
                nc.sync.dma_start(wqk_sb[:, 0, 512:D], wqk[:, 0, 512:D])
                for dc in range(1, DC):
                    nc.gpsimd.dma_start(wqk_sb[:, dc], wqk[:, dc])
                for dc in range(1, DC):
                    eng = nc.sync if dc % 2 else nc.scalar
                    eng.dma_start(xb0[:, dc], xqT[:, dc, 512:NQ])
                xb_q0 = load_x(xqT, 0, 512, split=4, engs=(nc.sync, nc.scalar))
                nc.gpsimd.dma_start(wv_rs[:, 0:4], wv[:, 0:4])
                nc.gpsimd.dma_start(wv_rs[:, 4:8], wv[:, 4:8])
                nc.gpsimd.dma_start(mask_a, mask_a_in[:, :])
                nc.gpsimd.dma_start(mask_b, mask_b_in[:, :])
                nc.gpsimd.dma_start(ones_sb, ones_in[:, :])
                for i in range(4):
                    nc.gpsimd.dma_start(
                        xv_sb[:, i * 4 : (i + 1) * 4, :],
                        xv_in[:, i * 4 : (i + 1) * 4, :],
                    )


                # ---- G^T phase: G = X_q @ (Wq Wk^T); qb=1 first so
                # attention slot 7 can start right after it ----
                for eh in range(2):
                    ps_list = [pp.tile([P, 512], F32, tag="ps", name=f"psk{eh}_{i}") for i in range(4)]
                    for dc in range(DC):
                        for i in range(4):
                            ec = eh * 4 + i
                            nc.tensor.matmul(
                                ps_list[i],
                                wqk_sb[:, dc, ec * P : (ec + 1) * P],
                                xb0[:, dc],
                                start=(dc == 0),
                                stop=(dc == DC - 1),
                            )
                    for i in range(4):
                        ec = eh * 4 + i
                        if i % 2 == 0:
                            nc.vector.tensor_copy(
                                qt_sb[:, ec, 512:NQ], ps_list[i]
                            )
                        else:
                            nc.scalar.copy(qt_sb[:, ec, 512:NQ], ps_list[i])
                xb = xb_q0
                # X_k^T resident (raw keys -- no K projection exists),
                # queued behind the qb=0 queries; X_v streams on SWDGE.
                for i in range(4):
                    eng = nc.sync if i % 2 else nc.scalar
                    eng.dma_start(
                        kt_sb[:, :, i * 512 : (i + 1) * 512],
                        xkT[:, :, i * 512 : (i + 1) * 512],
                    )
                for ec in range(EC):
                    ps = pp.tile([P, 512], F32, tag="ps")
                    for dc in range(DC):
                        nc.tensor.matmul(
                            ps,
                            wqk_sb[:, dc, ec * P : (ec + 1) * P],
                            xb[:, dc],
                            start=(dc == 0),
                            stop=(dc == DC - 1),
                        )
                    if ec % 2 == 0:
                        nc.vector.tensor_copy(qt_sb[:, ec, 0:512], ps)
                    else:
                        nc.scalar.copy(qt_sb[:, ec, 0:512], ps)

            # ================= attention phase =================
            # Scores are computed TRANSPOSED (scores^T[k, q], keys on
            # partitions): exp then lands directly in the attn^T layout the
            # attn@V matmul wants as stationary -- no PE transposes at all.
            # The softmax denominator is an ap=1 matmul against a ones vector
            # accumulated over key tiles (essentially free on the PE).
            with tc.tile_pool(name="attn", bufs=3) as attnp:
                # Software pipeline at group granularity: the den/attn@V
                # matmuls of group (s, g) are emitted after the NEXT group's
                # scores+exp, so the PE never sits waiting on an exp -- there
                # is always a ready matmul in program order.
                slot_state = {}
                pending = []

                def consume(s, g):
                    st = slot_state[s]
                    nt = st["nt"]
                    cnt = min(4, nt - g * 4)
                    for i in range(cnt):
                        t = g * 4 + i
                        nc.tensor.matmul(
                            st["ps_den"],
                            st["attnT"][:, t, :],
                            ones_sb,
                            start=(t == 0),
                            stop=(t == nt - 1),
                        )
                    if g == st["ng"] - 1:
                        # slot finished: U^T = Xv^T @ attn^T directly (d on
                        # partitions, Xv natural layout as stationary), then
                        # Y = U @ Wv with 1/den applied on copy-out.
                        rec = attnp.tile([P, 1], F32, tag="rec")
                        nc.vector.reciprocal(rec, st["ps_den"])
                        ut = attnp.tile([P, DC, P], BF16, tag="ut", name=f"ut{s}")
                        for dc in range(DC):
                            ps_u = pp.tile(
                                [P, P], F32, tag="utacc", bufs=3,
                                name=f"psu{s}_{dc}",
                            )
                            for t in range(nt):
                                nc.tensor.matmul(
                                    ps_u,
                                    xv_sb[:, t, dc * P : (dc + 1) * P],
                                    st["attnT"][:, t, :],
                                    start=(t == 0),
                                    stop=(t == nt - 1),
                                )
                            nc.vector.tensor_copy(ut[:, dc, :], ps_u)
                        out_sb = attnp.tile([P, D], BF16, tag="out", bufs=3)
                        for eh in range(2):
                            ps_y = pp.tile(
                                [P, 512], F32, tag="ps", bufs=4,
                                name=f"psy{eh}_{s}",
                            )
                            for dc in range(DC):
                                nc.tensor.matmul(
                                    ps_y,
                                    ut[:, dc, :],
                                    wv_rs[:, dc, eh * 512 : (eh + 1) * 512],
                                    start=(dc == 0),
                                    stop=(dc == DC - 1),
                                )
                            if eh == 0 and s == 0:
                                nc.scalar.activation(
                                    out=out_sb[:, 0:512],
                                    in_=ps_y,
                                    func=mybir.ActivationFunctionType.Copy,
                                    scale=rec,
                                )
                            else:
                                nc.vector.tensor_scalar_mul(
                                    out_sb[:, eh * 512 : (eh + 1) * 512],
                                    ps_y,
                                    rec,
                                )
                            nc.sync.dma_start(
                                out[s * P : (s + 1) * P, eh * 512 : (eh + 1) * 512],
                                out_sb[:, eh * 512 : (eh + 1) * 512],
                            )
                        del slot_state[s]

                for s in range(NSLOT - 1, -1, -1):
                    L = 256 * (s + 1)
                    nt = L // P
                    ng = (nt + 3) // 4
                    slot_state[s] = {
                        "nt": nt,
                        "ng": ng,
                        "attnT": attnp.tile(
                            [P, S // P, P], BF16, tag="attnT", bufs=3,
                            name=f"attnT{s}",
                        ),
                        "ps_den": pp.tile(
                            [P, 1], F32, tag="ps_den", bufs=1, name=f"psden{s}"
                        ),
                    }
                    for g in range(ng):
                        cnt = min(4, nt - g * 4)
                        psT = pp.tile([P, 512], F32, tag="ps")
                        for i in range(cnt):
                            t = g * 4 + i
                            for ec in range(EC):
                                nc.tensor.matmul(
                                    psT[:, i * P : (i + 1) * P],
                                    kt_sb[:, ec, t * P : (t + 1) * P],
                                    qt_sb[:, ec, s * P : (s + 1) * P],
                                    start=(ec == 0),
                                    stop=(ec == EC - 1),
                                )
                        if g == ng - 1:
                            # causal mask on the last two key tiles: for h=0
                            # mask_a is the triangle and mask_b is all -1e9;
                            # for h=1 mask_a is zero and mask_b the triangle.
                            nc.vector.tensor_add(
                                out=psT[:, (cnt - 2) * P : (cnt - 1) * P],
                                in0=psT[:, (cnt - 2) * P : (cnt - 1) * P],
                                in1=mask_a,
                            )
                            nc.vector.tensor_add(
                                out=psT[:, (cnt - 1) * P : cnt * P],
                                in0=psT[:, (cnt - 1) * P : cnt * P],
                                in1=mask_b,
                            )
                        nc.scalar.activation(
                            out=slot_state[s]["attnT"][:, g * 4 : g * 4 + cnt, :],
                            in_=psT[:, : cnt * P],
                            func=mybir.ActivationFunctionType.Exp,
                            scale=SCALE,
                        )
                        if pending:
                            consume(*pending.pop(0))
                        pending.append((s, g))
                while pending:
                    consume(*pending.pop(0))

    nc.compile()
    return nc


_NC_CACHE = None


def _get_nc():
    global _NC_CACHE
    if _NC_CACHE is None:
        _NC_CACHE = build_attention_nc()
    return _NC_CACHE


def _make_masks(h: int) -> tuple[np.ndarray, np.ndarray]:
    """Transposed masks [key kk, query r] for the last two key tiles."""
    import ml_dtypes

    kk = np.arange(P)[:, None]
    r = np.arange(P)[None, :]
    tri = np.where(kk <= r, 0.0, NEG).astype(np.float32)
    if h == 0:
        mask_a, mask_b = tri, np.full((P, P), NEG, dtype=np.float32)
    else:
        mask_a, mask_b = np.zeros((P, P), dtype=np.float32), tri
    return mask_a.astype(ml_dtypes.bfloat16), mask_b.astype(ml_dtypes.bfloat16)


def kernel(
    inputs_for_keys,
    inputs_for_values,
    inputs_for_queries,
    weight_K,
    weight_V,
    weight_Q,
    trace=False,
):
    import ml_dtypes

    bf16 = ml_dtypes.bfloat16

    def _xT(x):  # [rows, D] f32 -> [P, DC, rows] bf16 (transposed, p-major)
        xt = np.asarray(x, dtype=np.float32).T.reshape(DC, P, x.shape[0])
        return np.ascontiguousarray(xt.transpose(1, 0, 2)).astype(bf16)

    def _w(w):  # [D, D] f32 -> [P, DC, D] bf16 (d_in on partitions, p-major)
        wr = np.asarray(w, dtype=np.float32).reshape(DC, P, D)
        return np.ascontiguousarray(wr.transpose(1, 0, 2)).astype(bf16)

    xk_full = np.asarray(inputs_for_keys, dtype=np.float32)
    xv_full = np.asarray(inputs_for_values, dtype=np.float32)
    xq_full = np.asarray(inputs_for_queries, dtype=np.float32)

    w_v = _w(weight_V)
    w_qk = _w(
        np.asarray(weight_Q, dtype=np.float32)
        @ np.asarray(weight_K, dtype=np.float32).T
    )

    def _xv(x):  # [S, D] f32 -> [P, S//P, D] bf16 (keys on partitions)
        xr = np.asarray(x, dtype=np.float32).reshape(S // P, P, D)
        return np.ascontiguousarray(xr.transpose(1, 0, 2)).astype(bf16)

    xkT = [_xT(xk_full[b]) for b in range(B)]
    xv = [_xv(xv_full[b]) for b in range(B)]

    masks = [_make_masks(0), _make_masks(1)]
    ones_np = np.ones((P, 1), dtype=np.float32).astype(bf16)
    in_maps = []
    for c in range(2 * B):
        b, h = c // 2, c % 2
        rows = np.concatenate(
            [
                xq_full[b, 256 * s + 128 * h : 256 * s + 128 * h + P, :]
                for s in range(NSLOT)
            ],
            axis=0,
        )
        in_maps.append(
            {
                "xqT": _xT(rows),
                "xkT": xkT[b],
                "xv": xv[b],
                "wqk": w_qk,
                "wv": w_v,
                "mask_a": masks[h][0],
                "mask_b": masks[h][1],
                "ones": ones_np,
            }
        )

    nc = _get_nc()
    res = bass_utils.run_bass_kernel_spmd(
        nc, in_maps, core_ids=list(range(2 * B)), trace=trace
    )

    out = np.empty((B, S, D), dtype=np.float32)
    for c in range(2 * B):
        b, h = c // 2, c % 2
        o = np.asarray(res.results[c]["out"], dtype=np.float32)
        for s in range(NSLOT):
            out[b, 256 * s + 128 * h : 256 * s + 128 * h + P, :] = o[
                s * P : (s + 1) * P, :
            ]

    if trace:
        return out, res
    return out
